# revision 10
# baseline (speedup 1.0000x reference)
"""Trainium2 Bass kernel for nn_LiftedStructureLoss.

Strategy (data-parallel over rows, per sharding hint):
  - Host sorts rows by class label -> same-class pairs become a narrow
    band around the diagonal of the similarity matrix.
  - 8 cores each take 1024 sorted rows. Each core computes its
    [1024, 8192] slab of sim = Yblk @ Y^T with bf16 PE matmuls
    (f32 PSUM accumulate), and per row:
      total   = sum_j exp(sim[i,j])                  (fused ACT exp+rowsum)
      same    = sum_{same-class j} exp(sim[i,j])     (narrow window pass)
      pos_raw = sum_{same-class j} exp(1 - sim[i,j]) (narrow window pass)
  - Host combines: neg = total - same; pos = pos_raw - diag correction;
    loss = mean(log pos + log neg). Last-row mean pos/neg sim stats are
    O(n*d) and computed on host in f32.

The same-class "window" for the 128-row tile starting at sorted row
128*T is the column range [128*T - MARG, 128*T - MARG + W). Window
columns are materialized per (core, tile) on the host (zero-padded at
the edges) so all device addressing is static; a -1e30 additive mask
(bm) kills non-same-class columns inside exp().
"""

import numpy as np
import ml_dtypes

N = 8192
D = 512
NCORES = 8
ROWS_PER_CORE = N // NCORES          # 1024
TILES_PER_CORE = ROWS_PER_CORE // 128  # 8
NEG_BIG = -1.0e30

_PROG_CACHE = {}


def _build_program(W):
    import concourse.bass as bass  # noqa: F401
    import concourse.mybir as mybir
    import concourse.tile as tile
    from concourse import bacc

    dt = mybir.dt
    Alu = mybir.AluOpType
    Act = mybir.ActivationFunctionType

    nc = bacc.Bacc()
    ybt_d = nc.declare_dram_parameter("ybt", [4, 128, ROWS_PER_CORE], dt.bfloat16, False)
    yt_d = nc.declare_dram_parameter("yt", [4, 128, N], dt.bfloat16, False)
    win_d = nc.declare_dram_parameter("win", [TILES_PER_CORE, 4, 128, W], dt.bfloat16, False)
    bm_d = nc.declare_dram_parameter("bm", [TILES_PER_CORE, 128, W], dt.float32, False)
    stats_d = nc.declare_dram_parameter("stats", [TILES_PER_CORE, 128, 3], dt.float32, True)

    with tile.TileContext(nc) as tc:
        with (
            tc.tile_pool(name="persist", bufs=1) as persist,
            tc.tile_pool(name="wpool", bufs=2) as wpool,
            tc.tile_pool(name="epool", bufs=3) as epool,
            tc.tile_pool(name="psA", bufs=3, space="PSUM") as psA,
            tc.tile_pool(name="psW", bufs=2, space="PSUM") as psW,
        ):
            # Resident Y^T [512, 8192] as 4 k-chunks x 8 column chunks.
            # One tile per DMA: a tile with many DMA writers on distinct
            # queues would exceed the per-instruction sync-wait limit on
            # the first matmul that reads it.
            yts = []
            for k in range(4):
                row = []
                for p in range(8):
                    tk = persist.tile([128, 1024], dt.bfloat16, tag=f"yt{k}_{p}")
                    nc.sync.dma_start(tk[:], yt_d[k, :, p * 1024:(p + 1) * 1024])
                    row.append(tk)
                yts.append(row)
            ybts = []
            for k in range(4):
                tk = persist.tile([128, ROWS_PER_CORE], dt.bfloat16, tag=f"ybt{k}")
                nc.sync.dma_start(tk[:], ybt_d[k])
                ybts.append(tk)

            for t in range(TILES_PER_CORE):
                wts = []
                for k in range(4):
                    wk = wpool.tile([128, W], dt.bfloat16, tag=f"wt{k}")
                    nc.sync.dma_start(wk[:], win_d[t, k])
                    wts.append(wk)
                bmt = wpool.tile([128, W], dt.float32, tag="bmt")
                nc.sync.dma_start(bmt[:], bm_d[t])
                acc = wpool.tile([128, 8], dt.float32, tag="acc")
                stt = wpool.tile([128, 3], dt.float32, tag="stt")

                # ---- main pass: 8 chunks of 1024 columns ----
                for c8 in range(8):
                    pm = psA.tile([128, 1024], dt.float32, tag="pm")
                    for half in range(2):
                        for k in range(4):
                            nc.tensor.matmul(
                                pm[:, half * 512:half * 512 + 512],
                                ybts[k][:, t * 128:(t + 1) * 128],
                                yts[k][c8][:, half * 512:half * 512 + 512],
                                start=(k == 0),
                                stop=(k == 3),
                            )
                    esc = epool.tile([128, 1024], dt.float32, tag="esc")
                    nc.scalar.activation(
                        esc[:], pm[:], Act.Exp, accum_out=acc[:, c8:c8 + 1]
                    )

                # ---- window pass ----
                pw = psW.tile([128, W], dt.float32, tag="pw")
                for k in range(4):
                    nc.tensor.matmul(
                        pw[:],
                        ybts[k][:, t * 128:(t + 1) * 128],
                        wts[k][:],
                        start=(k == 0),
                        stop=(k == 3),
                    )
                tneg = wpool.tile([128, W], dt.float32, tag="tneg")
                nc.vector.scalar_tensor_tensor(
                    tneg[:], pw[:], -1.0, bmt[:], op0=Alu.mult, op1=Alu.add
                )
                tsame = wpool.tile([128, W], dt.float32, tag="tsame")
                nc.vector.scalar_tensor_tensor(
                    tsame[:], pw[:], 1.0, bmt[:], op0=Alu.mult, op1=Alu.add
                )
                e1 = epool.tile([128, W], dt.float32, tag="e1")
                nc.scalar.activation(
                    e1[:], tneg[:], Act.Exp, bias=1.0,
                    accum_out=stt[:, 2:3],
                )
                e2 = epool.tile([128, W], dt.float32, tag="e2")
                nc.scalar.activation(
                    e2[:], tsame[:], Act.Exp,
                    accum_out=stt[:, 1:2],
                )
                nc.vector.tensor_reduce(
                    stt[:, 0:1], acc[:],
                    axis=mybir.AxisListType.X, op=Alu.add,
                )
                nc.sync.dma_start(stats_d[t], stt[:])
    nc.finalize()
    return nc


def _get_program(W):
    if W not in _PROG_CACHE:
        _PROG_CACHE[W] = _build_program(W)
    return _PROG_CACHE[W]


def _prepare(inputs, targets):
    x = np.ascontiguousarray(np.asarray(inputs, dtype=np.float32))
    t = np.asarray(targets).astype(np.int64).reshape(-1)
    assert x.shape == (N, D), x.shape

    perm = np.argsort(t, kind="stable")
    ts_ = t[perm]
    Y = x[perm]

    # window geometry: margin must cover the largest class
    cnt = np.bincount(ts_, minlength=1)
    maxc = int(cnt.max())
    marg = 128
    while marg < maxc - 1:
        marg += 128
    W = 128 + 2 * marg

    YT = np.ascontiguousarray(Y.T)                       # [D, N] f32
    yt_bf = YT.reshape(4, 128, N).astype(ml_dtypes.bfloat16)

    # zero-padded Y^T for window slicing: padded col g_pad = g + marg
    YTp = np.zeros((D, N + 2 * marg + 128), dtype=np.float32)
    YTp[:, marg:marg + N] = YT
    YTp_bf = YTp.astype(ml_dtypes.bfloat16).reshape(4, 128, -1)

    in_maps = []
    for c in range(NCORES):
        ybt = np.ascontiguousarray(
            yt_bf[:, :, c * ROWS_PER_CORE:(c + 1) * ROWS_PER_CORE]
        )
        win = np.empty((TILES_PER_CORE, 4, 128, W), dtype=ml_dtypes.bfloat16)
        bm = np.empty((TILES_PER_CORE, 128, W), dtype=np.float32)
        for tt in range(TILES_PER_CORE):
            T = c * TILES_PER_CORE + tt
            start = 128 * T - marg                      # global col of window col 0
            win[tt] = YTp_bf[:, :, start + marg:start + marg + W]
            gcols = start + np.arange(W)
            valid = (gcols >= 0) & (gcols < N)
            ccls = np.where(valid, ts_[np.clip(gcols, 0, N - 1)], -1)
            rcls = ts_[128 * T:128 * T + 128]
            bm[tt] = np.where(rcls[:, None] == ccls[None, :], 0.0, NEG_BIG).astype(np.float32)
        in_maps.append({"ybt": ybt, "yt": yt_bf, "win": win, "bm": bm})

    # containment check: every row's class block within its window
    cstart = np.searchsorted(ts_, np.arange(ts_.max() + 1), side="left")
    cend = np.searchsorted(ts_, np.arange(ts_.max() + 1), side="right")
    rows = np.arange(N)
    Trow = rows // 128
    assert (cstart[ts_] >= 128 * Trow - marg).all(), "window underflow"
    assert (cend[ts_] <= 128 * Trow - marg + W).all(), "window overflow"

    return x, t, perm, ts_, Y, W, in_maps


def _run_device(W, in_maps, trace=False):
    from concourse.bass_utils import run_bass_kernel_spmd

    nc = _get_program(W)
    return run_bass_kernel_spmd(
        nc, in_maps, list(range(NCORES)), trace=trace,
        trace_cores=[0] if trace else None,
    )


def _combine(x, t, perm, ts_, Y, res):
    f32 = np.float32
    total_s = np.empty(N, f32)
    same_s = np.empty(N, f32)
    pos_raw_s = np.empty(N, f32)
    for c in range(NCORES):
        st = np.asarray(res[c]["stats"])        # [tiles, 128, 3]
        sl = slice(ROWS_PER_CORE * c, ROWS_PER_CORE * (c + 1))
        total_s[sl] = st[:, :, 0].reshape(-1)
        same_s[sl] = st[:, :, 1].reshape(-1)
        pos_raw_s[sl] = st[:, :, 2].reshape(-1)

    # diagonal handling: reference keeps self-sim in pos iff sim_ii < 1.
    sii = np.einsum("ij,ij->i", Y, Y).astype(f32)
    dcorr = np.where(sii >= 1.0, np.exp((f32(1.0) - sii)), f32(0.0)).astype(f32)
    pos_sum = pos_raw_s - dcorr
    neg_sum = total_s - same_s

    cnt = np.bincount(ts_, minlength=1)
    has_neg = cnt[ts_] < N                               # rows with any negative
    pos_loss = np.log(pos_sum)
    neg_loss = np.log(neg_sum)
    loss = f32(np.sum(np.where(has_neg, pos_loss + neg_loss, f32(0.0)), dtype=np.float32) / f32(N))
    prec = f32(np.mean((1.0 - has_neg.astype(np.float32)).astype(np.float32)))

    # last-row stats (original row order), O(n*d) on host in f32
    s_last = (x[-1:] @ x.T)[0].astype(f32)
    lp = (t == t[-1]) & (s_last < 1.0)
    ln = t != t[-1]
    mps = f32(np.sum(np.where(lp, s_last, f32(0.0)), dtype=np.float32) / f32(lp.sum()))
    mns = f32(np.sum(np.where(ln, s_last, f32(0.0)), dtype=np.float32) / f32(ln.sum()))
    return loss, prec, mps, mns


def kernel(inputs, targets, nonorm):
    x, t, perm, ts_, Y, W, in_maps = _prepare(inputs, targets)
    out = _run_device(W, in_maps, trace=False)
    loss, prec, mps, mns = _combine(x, t, perm, ts_, Y, out.results)
    return (np.float32(loss), np.float32(prec), np.float32(mps), np.float32(mns))
